# revision 25
# baseline (speedup 1.0000x reference)
"""Trainium2 Bass kernel: per-batch-row stable partition (facts first, pads last).

For each batch row b: out[b] = sentout[b][order] where order lists positions
with nl_input[b] != 0 first (original order), then positions == 0.

Strategy (pure data parallel over B=16 on 8 cores, 2 rows/core):
  - On device, compute for every source position l its destination row
      dest[l] = l - C[l]            if nl[l] != 0   (C = incl. cumsum of is_pad)
      dest[l] = F + C[l] - 1        if nl[l] == 0   (F = #non-pads in the row)
    via one PE transpose + two triangular-ones matmuls + a free-dim scan,
    all kept off gpsimd so Q7 is free for scatter descriptor generation.
  - Stream sentout through SBUF (strided loads on both HWDGE queues; small
    leading units so scatters start early) and scatter rows to their
    destinations with indirect DMA (4KB descriptors, one per row).
  - Scatter->scatter WAW sync deps are downgraded to engine-order-only
    (destinations are a permutation, hence disjoint); without this Tile
    serializes every scatter on the previous one's completion.
HW exec ~94.4us/core; DMA aggregate sits at ~420 GB/s (fabric wall) for
the whole data window, within ~2-3us of the achievable floor.
"""

import numpy as np

import concourse.bass as bass
import concourse.mybir as mybir
import concourse.tile as tile
from concourse.bacc import Bacc
from concourse.bass_utils import run_bass_kernel_spmd
from concourse.masks import make_identity, make_upper_triangular

B, L, D = 16, 2048, 1024
NCORES = 8
BLOC = B // NCORES          # batch rows per core = 2
P = 128                     # SBUF partitions
NCHUNK = L // P             # 16 chunks of 128 rows per batch row
NCOL = BLOC * NCHUNK        # 32 columns in the index layout

_NC_CACHE = None


def _build_nc():
    f32 = mybir.dt.float32
    i32 = mybir.dt.int32
    Op = mybir.AluOpType

    nc = Bacc()
    sent = nc.declare_dram_parameter("sent", [BLOC * L, D], f32, isOutput=False)
    nl = nc.declare_dram_parameter("nl", [NCOL, P], i32, isOutput=False)
    out = nc.declare_dram_parameter("out", [BLOC * L, D], f32, isOutput=True)

    with tile.TileContext(nc) as tc:
        with (
            tc.tile_pool(name="const", bufs=1) as cpool,
            tc.tile_pool(name="idx", bufs=1) as ipool,
            tc.tile_pool(name="psum", bufs=1, space="PSUM") as ppool,
            tc.tile_pool(name="data", bufs=10) as dpool,
        ):
            # ---- index pipeline (kept off gpsimd: Q7 must be free to start
            # generating scatter descriptors as early as possible) ----
            # nl arrives as [NCOL, P]: partition j = b*16 + c holds
            # l = c*128 + p along the free dim. Tiny DMA, issued first.
            nl_t = ipool.tile([NCOL, P], i32)
            nc.sync.dma_start(nl_t[:], nl[:])

            # ---- constants (gpsimd, but only ~1us total and traced before
            # any scatter, so Q7 is free again well before descriptors) ----
            ut = cpool.tile([P, P], f32)          # ut[q,p] = 1 iff q <= p
            make_upper_triangular(nc, ut[:], val=1.0, diag=True)
            ones = cpool.tile([P, P], f32)
            nc.gpsimd.memset(ones[:], 1.0)
            ident = cpool.tile([NCOL, NCOL], f32)
            make_identity(nc, ident[:])
            liota = cpool.tile([P, NCHUNK], i32)  # l = p + 128*c
            nc.gpsimd.iota(liota[:], [[P, NCHUNK]], base=0, channel_multiplier=1)
            lf = cpool.tile([P, NCHUNK], f32)
            nc.vector.tensor_copy(lf[:], liota[:])

            nl_f = ipool.tile([NCOL, P], f32)
            nc.vector.tensor_copy(nl_f[:], nl_t[:])
            ispad_pm = ipool.tile([NCOL, P], f32)
            nc.vector.tensor_scalar(ispad_pm[:], nl_f[:], 0.0, None, Op.is_equal)

            # transpose to column-major: ispad[p, b*16+c] for l = c*128 + p
            ps_t = ppool.tile([P, NCOL], f32)
            nc.tensor.transpose(ps_t[:], ispad_pm[:], ident[:])
            ispad = ipool.tile([P, NCOL], f32)
            nc.vector.tensor_copy(ispad[:], ps_t[:])

            # within-column inclusive cumsum (over partitions) + column sums
            cw_ps = ppool.tile([P, NCOL], f32)
            nc.tensor.matmul(cw_ps[:], lhsT=ut[:], rhs=ispad[:], start=True, stop=True)
            s_ps = ppool.tile([P, NCOL], f32)
            nc.tensor.matmul(s_ps[:], lhsT=ones[:], rhs=ispad[:], start=True, stop=True)
            s_sb = ipool.tile([P, NCOL], f32)
            nc.vector.tensor_copy(s_sb[:], s_ps[:])

            dest_all = ipool.tile([P, NCOL], i32)
            for b in range(BLOC):
                blk = slice(b * NCHUNK, (b + 1) * NCHUNK)
                # inclusive prefix of column sums along the 16 chunks
                incl = ipool.tile([P, NCHUNK], f32, tag="incl")
                nc.vector.tensor_tensor_scan(
                    incl[:], s_sb[:, blk], s_sb[:, blk], 0.0, Op.add, Op.bypass
                )
                # exclusive chunk prefix
                excl = ipool.tile([P, NCHUNK], f32, tag="excl")
                nc.vector.tensor_tensor(
                    out=excl[:], in0=incl[:], in1=s_sb[:, blk], op=Op.subtract
                )
                # C = inclusive cumsum of is_pad over l
                cfull = ipool.tile([P, NCHUNK], f32, tag="cfull")
                nc.vector.tensor_tensor(
                    out=cfull[:], in0=cw_ps[:, blk], in1=excl[:], op=Op.add
                )
                # fact destination: l - C (+ row base)
                t1 = ipool.tile([P, NCHUNK], f32, tag="t1")
                nc.vector.tensor_tensor(out=t1[:], in0=lf[:], in1=cfull[:], op=Op.subtract)
                if b:
                    nc.vector.tensor_scalar_add(t1[:], t1[:], float(b * L))
                # pad destination: (L - T) + C - 1 (+ row base); T = incl[:, -1]
                t2 = ipool.tile([P, NCHUNK], f32, tag="t2")
                nc.vector.tensor_tensor(
                    out=t2[:],
                    in0=cfull[:],
                    in1=incl[:, NCHUNK - 1 : NCHUNK].to_broadcast([P, NCHUNK]),
                    op=Op.subtract,
                )
                nc.vector.tensor_scalar_add(t2[:], t2[:], float(L - 1 + b * L))
                # blend: dest = t1 + is_pad * (t2 - t1)   (exact small ints in f32)
                destf = ipool.tile([P, NCHUNK], f32, tag="destf")
                nc.vector.tensor_tensor(out=t2[:], in0=t2[:], in1=t1[:], op=Op.subtract)
                nc.vector.tensor_tensor(out=t2[:], in0=t2[:], in1=ispad[:, blk], op=Op.mult)
                nc.vector.tensor_tensor(out=destf[:], in0=t1[:], in1=t2[:], op=Op.add)
                nc.vector.tensor_copy(dest_all[:, blk], destf[:])

            # ---- data movement: 8 x (2MB strided load + 512-row scatter) ----
            # Each load packs K=4 row-groups into the free dim: dtile[p, i*D:(i+1)*D]
            # holds sent row (c0+i)*128+p; the scatter pairs offset (p, i) with
            # that 4KB chunk (indices ravel partition-major).
            # Unit sizes: small leading units so the first scatters start ~4us
            # earlier (the window otherwise runs load-only until the first
            # 2MB load lands); 2MB units after that for low per-DMA overhead.
            KS = [1, 1, 2, 4, 4, 4, 4, 4, 4, 4]
            assert sum(KS) == NCOL
            scatter_names = set()
            c0 = 0
            for i, K in enumerate(KS):
                dtile = dpool.tile([P, 4 * D], f32, tag="dtile")
                src = sent[c0 * P : (c0 + K) * P, :].rearrange("(g p) d -> p g d", p=P)
                eng = nc.sync if i % 2 == 0 else nc.scalar
                eng.dma_start(
                    dtile[:, : K * D].rearrange("p (g d) -> p g d", g=K), src
                )
                for j in range(K):
                    sc = nc.gpsimd.indirect_dma_start(
                        out=out[:],
                        out_offset=bass.IndirectOffsetOnAxis(
                            ap=dest_all[:, c0 + j : c0 + j + 1], axis=0
                        ),
                        in_=dtile[:, j * D : (j + 1) * D],
                        in_offset=None,
                    )
                    # The 32 scatters write pairwise-disjoint row sets of `out`
                    # (dest is a permutation), so the WAW completion-waits Tile
                    # inserts between them are spurious and serialize the whole
                    # scatter stream. Downgrade scatter->scatter sync edges to
                    # engine-order-only.
                    mi = sc.ins
                    for dep in mi.sync_dependency_names():
                        if dep in scatter_names:
                            mi.remove_dependency(
                                dep, mybir.DependencyInfo.SYNC_ONLY
                            )
                            mi.add_dependency(
                                dep, mybir.DependencyInfo.NO_SYNC_ONLY
                            )
                    scatter_names.add(mi.name)
                c0 += K
    nc.compile()
    return nc


def _get_nc():
    global _NC_CACHE
    if _NC_CACHE is None:
        _NC_CACHE = _build_nc()
    return _NC_CACHE


def _make_in_maps(sentout, nl_input):
    sent = np.ascontiguousarray(np.asarray(sentout, dtype=np.float32)).reshape(
        NCORES, BLOC * L, D
    )
    nl = np.ascontiguousarray(np.asarray(nl_input).astype(np.int32)).reshape(
        NCORES, NCOL, P
    )
    return [{"sent": sent[c], "nl": nl[c]} for c in range(NCORES)]


def run_on_device(sentout, nl_input, **kwargs):
    """Run the Bass kernel; returns (full_output, BassKernelResults)."""
    nc = _get_nc()
    res = run_bass_kernel_spmd(
        nc, _make_in_maps(sentout, nl_input), core_ids=list(range(NCORES)), **kwargs
    )
    outs = [r["out"].reshape(BLOC, L, D) for r in res.results]
    return np.concatenate(outs, axis=0), res


def kernel(sentout, nl_input):
    out, _ = run_on_device(sentout, nl_input)
    return out
